# revision 16
# baseline (speedup 1.0000x reference)
"""CrissCrossAttention Trainium2 kernel (v2: all-bf16, pipelined phases).

Math notes (verified in float64): the reference's column-attention einsum
('bnjid,bnkid->bnjik' applied to grid-swapped q/k/v) is an alpha-renaming
that exactly undoes the swap, so reference == 2 * row_attention:
    out = (2 * row_attn(x)) @ Wo + bo
Row attention per (batch, head, grid-row i):
    S = Q_i K_i^T * d^-0.5 ; P = softmax_k(S) ; O_i = P V_i
with grid 64x64 (n = 4096 = i*64 + j), heads=8, d=64.

Distribution: data-parallel over batch; core b handles x[b].

v2 changes vs v1 (279us -> target ~190us):
  - whole PE datapath in bf16 (x cast once; transposes 1cyc/row; all
    LDWEIGHTS FWL-eligible 128-col or cheap 64-col)
  - pool scoping so phase-B SBUF never reuses phase-A-hot space: no
    cross-phase barrier, PE stream stays dense, HAM stays warm
  - softmax chain spread over ACT/DVE/GpSimd (P-mul on GpSimd)
  - deeper software skew (3) between S-front and O-tail
  - ot in bf16 (was f32r): final GEMM LDW 53ns instead of ~200ns
"""

import sys

if "/opt/trn_rl_repo" not in sys.path:
    sys.path.insert(0, "/opt/trn_rl_repo")

import numpy as np

import concourse.bass as bass
import concourse.mybir as mybir
import concourse.tile as tile
from concourse import bacc
from concourse.masks import make_identity

F32 = mybir.dt.float32
BF16 = mybir.dt.bfloat16

N = 4096
D = 512
G = 64          # grid side
NC = 4          # dim chunks of 128 (2 heads each)
NT = 32         # n tiles of 128
NB = 8          # n banks of 512
SCALE = G ** -0.5


def build_kernel(n_cores: int = 8):
    nc = bacc.Bacc("TRN2", target_bir_lowering=False, debug=False,
                   num_devices=n_cores)

    x_d = nc.dram_tensor("x", [N, D], F32, kind="ExternalInput").ap()
    wq_d = nc.dram_tensor("Wq", [D, D], F32, kind="ExternalInput").ap()
    wk_d = nc.dram_tensor("Wk", [D, D], F32, kind="ExternalInput").ap()
    wv_d = nc.dram_tensor("Wv", [D, D], F32, kind="ExternalInput").ap()
    wo_d = nc.dram_tensor("Wo", [D, D], F32, kind="ExternalInput").ap()
    bo_d = nc.dram_tensor("bo", [D], F32, kind="ExternalInput").ap()
    out_d = nc.dram_tensor("out", [N, D], F32, kind="ExternalOutput").ap()

    with tile.TileContext(nc) as tc:
        with (
            tc.tile_pool(name="consts", bufs=1) as consts,
            tc.tile_pool(name="qk", bufs=1) as qkpool,
            tc.tile_pool(name="vpool", bufs=1) as vpool,
            tc.tile_pool(name="wbf", bufs=1) as wbf,
        ):
            ident_bf = consts.tile([128, 128], BF16, tag="idb")
            make_identity(nc, ident_bf)

            qT = [qkpool.tile([128, N], BF16, tag=f"qT{c}", name=f"qT{c}")
                  for c in range(NC)]
            kT = [qkpool.tile([128, N], BF16, tag=f"kT{c}", name=f"kT{c}")
                  for c in range(NC)]
            v_sb = vpool.tile([128, NT, D], BF16, tag="v")

            wq_bf = wbf.tile([128, NC, D], BF16, tag="wq")
            wk_bf = wbf.tile([128, NC, D], BF16, tag="wk")
            wv_bf = wbf.tile([128, NC, D], BF16, tag="wv")
            wo_bf = wbf.tile([128, NC, D], BF16, tag="wo")
            bo128 = wbf.tile([128, D], F32, tag="bo")

            # ---- phase A: x load/cast/transpose + projections ----------
            with (
                tc.tile_pool(name="xt", bufs=1) as xtpool,
            ):
                xT = xtpool.tile([128, NC, N], BF16, tag="xT")

                with (
                    tc.tile_pool(name="wraw", bufs=1) as wraw,
                    tc.tile_pool(name="xin", bufs=6) as xin,
                    tc.tile_pool(name="xbf", bufs=6) as xbfp,
                    tc.tile_pool(name="psum0", bufs=1, space="PSUM") as psum,
                ):
                    # weights: fp32 staging on the fast sync HWDGE queue,
                    # issued before the x tiles; bf16 casts are emitted
                    # mid-queue (not head-of-line) just before first use.
                    # W staging on the gpsimd SWDGE queue (parallel to the
                    # sync queue carrying x). Issue wq ALONE first so it
                    # completes ~3us in (all-at-once would finish together
                    # at ~12us and stall the bank-0 projections); wv/wo are
                    # deferred until mid-phase A.
                    wstages = [wraw.tile([128, NC, D], F32, tag=f"wstage{i}",
                                         name=f"wstage{i}")
                               for i in range(4)]
                    wds = (wq_d, wk_d, wv_d, wo_d)

                    def w_load(i):
                        nc.gpsimd.dma_start(
                            out=wstages[i],
                            in_=wds[i].rearrange("(kc p) e -> p kc e", p=128))

                    w_load(0)
                    w_load(1)
                    nc.sync.dma_start(
                        out=bo128,
                        in_=bass.AP(tensor=bo_d.tensor, offset=bo_d.offset,
                                    ap=[[0, 128], [1, D]]))

                    # x: load fp32 -> cast bf16 -> PE transpose -> xT
                    def load_bank(nb):
                        for nt in range(nb * 4, nb * 4 + 4):
                            xtile = xin.tile([128, D], F32, tag="xtile",
                                             name=f"xtile{nt}")
                            nc.sync.dma_start(
                                out=xtile,
                                in_=x_d[nt * 128:(nt + 1) * 128, :])
                            xb = xbfp.tile([128, D], BF16, tag="xbf",
                                           name=f"xbf{nt}")
                            if nt % 2 == 0:
                                nc.vector.tensor_copy(out=xb, in_=xtile)
                            else:
                                nc.scalar.copy(out=xb, in_=xtile)
                            tp = psum.tile([128, NC, 128], BF16, tag="xtp",
                                           bufs=3, name=f"xtp{nt}")
                            for kc in range(NC):
                                nc.tensor.transpose(
                                    tp[:, kc, :],
                                    xb[:, kc * 128:(kc + 1) * 128],
                                    ident_bf)
                            for kc in range(NC):
                                if kc % 2 == 0:
                                    nc.vector.tensor_copy(
                                        out=xT[:, kc,
                                               nt * 128:(nt + 1) * 128],
                                        in_=tp[:, kc, :])
                                else:
                                    nc.scalar.copy(
                                        out=xT[:, kc,
                                               nt * 128:(nt + 1) * 128],
                                        in_=tp[:, kc, :])

                    def qk_bank(nb):
                        for wsb, dst in ((wq_bf, qT), (wk_bf, kT)):
                            for c in range(NC):
                                pj = psum.tile([128, D], F32, tag="pj",
                                               bufs=4, name=f"pj{nb}_{c}")
                                for kc in range(NC):
                                    nc.tensor.matmul(
                                        pj,
                                        wsb[:, kc, c * 128:(c + 1) * 128],
                                        xT[:, kc, nb * D:(nb + 1) * D],
                                        start=(kc == 0), stop=(kc == NC - 1))
                                if c % 2 == 0:
                                    nc.vector.tensor_copy(
                                        out=dst[c][:, nb * D:(nb + 1) * D],
                                        in_=pj)
                                else:
                                    nc.scalar.copy(
                                        out=dst[c][:, nb * D:(nb + 1) * D],
                                        in_=pj)

                    # 1-bank software skew: bank nb+1's transposes land
                    # while the PE projects bank nb; v projections at the
                    # end (their consumers run last in phase B). W casts
                    # drop into the engine queues right when needed.
                    load_bank(0)
                    nc.vector.tensor_copy(out=wq_bf, in_=wstages[0])
                    nc.scalar.copy(out=wk_bf, in_=wstages[1])
                    for nb in range(NB):
                        if nb + 1 < NB:
                            load_bank(nb + 1)
                        if nb == 1:
                            w_load(2)
                        if nb == 2:
                            w_load(3)
                            nc.vector.tensor_copy(out=wv_bf, in_=wstages[2])
                        if nb == 4:
                            # fold the criss-cross 2x into Wo
                            nc.scalar.mul(out=wo_bf, in_=wstages[3], mul=2.0)
                        qk_bank(nb)
                    for nt in range(NT):
                        pv = psum.tile([128, D], F32, tag="pj", bufs=4,
                                       name=f"pv{nt}")
                        for kc in range(NC):
                            nc.tensor.matmul(
                                pv,
                                xT[:, kc, nt * 128:(nt + 1) * 128],
                                wv_bf[:, kc, :],
                                start=(kc == 0), stop=(kc == NC - 1))
                        if nt % 2 == 0:
                            nc.vector.tensor_copy(out=v_sb[:, nt, :],
                                                  in_=pv)
                        else:
                            nc.scalar.copy(out=v_sb[:, nt, :], in_=pv)

                # ---- phase B: attention + output projection ------------
                with (
                    tc.tile_pool(name="otpool", bufs=1) as otpool,
                    tc.tile_pool(name="attn", bufs=1) as attn,
                    tc.tile_pool(name="outsb", bufs=2) as outp,
                    tc.tile_pool(name="psumA", bufs=1, space="PSUM") as psumb,
                ):
                    ot = [otpool.tile([128, N], BF16, tag=f"ot{c}",
                                      name=f"ot{c}")
                          for c in range(NC)]

                    pbs = {}

                    def front(bk, c):
                        """S matmuls + softmax chain; result in pbs."""
                        sbt = psumb.tile([128, 8, G], F32, tag="sb", bufs=3,
                                         name=f"sb{bk}_{c}")
                        for s in range(8):
                            i = bk * 8 + s
                            for h in range(2):
                                p0 = h * 64
                                nc.tensor.matmul(
                                    sbt[p0:p0 + 64, s, :],
                                    qT[c][p0:p0 + 64, i * G:(i + 1) * G],
                                    kT[c][p0:p0 + 64, i * G:(i + 1) * G],
                                    start=True, stop=True,
                                    tile_position=(p0, p0))
                        eb = attn.tile([128, 8, G], BF16, tag="eb", bufs=4,
                                       name=f"eb{bk}_{c}")
                        nc.scalar.activation(
                            out=eb, in_=sbt,
                            func=mybir.ActivationFunctionType.Exp,
                            scale=SCALE)
                        sums = attn.tile([128, 8], F32, tag="sums", bufs=4,
                                         name=f"sums{bk}_{c}")
                        nc.vector.reduce_sum(out=sums, in_=eb,
                                             axis=mybir.AxisListType.X)
                        rec = attn.tile([128, 8], F32, tag="rec", bufs=4,
                                        name=f"rec{bk}_{c}")
                        nc.vector.reciprocal(out=rec, in_=sums)
                        rec_b = bass.AP(tensor=rec.tensor, offset=rec.offset,
                                        ap=[rec.ap[0], rec.ap[1], [0, G]])
                        pb = attn.tile([128, 8, G], BF16, tag="pb", bufs=4,
                                       name=f"pb{bk}_{c}")
                        # last units: DVE mul (faster) shortens the drain
                        # chain at the end of the kernel when nothing is
                        # left to hide it behind
                        if bk * NC + c >= NB * NC - 4:
                            nc.vector.tensor_mul(pb, eb, rec_b)
                        else:
                            nc.gpsimd.tensor_mul(pb, eb, rec_b)
                        pbs[(bk, c)] = pb

                    def tail(bk, c):
                        """P transpose, O matmuls, extract into OT."""
                        pb = pbs.pop((bk, c))
                        ptp = psumb.tile([128, 4, 128], BF16, tag="ptp",
                                         bufs=2, name=f"ptp{bk}_{c}")
                        for p2 in range(4):
                            nc.tensor.transpose(
                                ptp[:, p2, :], pb[:, 2 * p2:2 * p2 + 2, :],
                                ident_bf)
                        pts = attn.tile([128, 4, 128], BF16, tag="pts",
                                        bufs=4, name=f"pts{bk}_{c}")
                        if c % 2 == 0:
                            nc.vector.tensor_copy(out=pts, in_=ptp)
                        else:
                            nc.scalar.copy(out=pts, in_=ptp)

                        ob = psumb.tile([128, 2, 4, 128], F32, tag="ob",
                                        name=f"ob{bk}_{c}", bufs=1)
                        for p2 in range(4):
                            for e in range(2):
                                i = bk * 8 + 2 * p2 + e
                                nc.tensor.matmul(
                                    ob[:, e, p2, :],
                                    v_sb[e * 64:e * 64 + 64, i // 2,
                                         c * 128:(c + 1) * 128],
                                    pts[e * 64:e * 64 + 64, p2, :],
                                    start=True, stop=True,
                                    tile_position=(e * 64, 0))
                        # extract diagonal (per-head) blocks into OT
                        pstride = ot[c].ap[0][0]
                        for h in range(2):
                            src_ap = ob[h * 64:h * 64 + 64, :, :,
                                        h * 64:h * 64 + 64]
                            dst = bass.AP(
                                tensor=ot[c].tensor,
                                offset=(ot[c].offset + h * 64 * pstride
                                        + bk * 8 * G),
                                ap=[[pstride, 64], [G, 2], [2 * G, 4],
                                    [1, G]])
                            if h == 1:
                                nc.scalar.copy(out=dst, in_=src_ap)
                            else:
                                nc.vector.tensor_copy(out=dst, in_=src_ap)

                    def final_bank(bk):
                        """Output projection for this bank's 4 n-tiles."""
                        for nt in range(bk * 4, bk * 4 + 4):
                            fp = psumb.tile([128, D], F32, tag="fp", bufs=1,
                                            name=f"fp{nt}")
                            for c in range(NC):
                                nc.tensor.matmul(
                                    fp, ot[c][:, nt * 128:(nt + 1) * 128],
                                    wo_bf[:, c, :],
                                    start=(c == 0), stop=(c == NC - 1))
                            osb = outp.tile([128, D], F32, tag="osb",
                                            name=f"osb{nt}")
                            nc.vector.tensor_add(osb, fp, bo128)
                            nc.sync.dma_start(
                                out=out_d[nt * 128:(nt + 1) * 128, :],
                                in_=osb)

                    # software skew: S-matmuls run SKEW units ahead of the
                    # P-transpose/O-matmul tail so the PE never stalls on
                    # the ACT/DVE/GpSimd softmax chain
                    SKEW = 3
                    units = [(bk, c) for bk in range(NB) for c in range(NC)]
                    for idx in range(len(units) + SKEW):
                        if idx < len(units):
                            front(*units[idx])
                        if idx >= SKEW:
                            bk, c = units[idx - SKEW]
                            tail(bk, c)
                            if c == NC - 1:
                                final_bank(bk)

    nc.compile()
    return nc


_CACHED = None


def _get_nc():
    global _CACHED
    if _CACHED is None:
        _CACHED = build_kernel()
    return _CACHED


def run(inputs: dict, trace: bool = False):
    from concourse.bass_utils import run_bass_kernel_spmd
    nc = _get_nc()
    x = np.ascontiguousarray(inputs["x"], dtype=np.float32)
    b = x.shape[0]
    shared = {k: np.ascontiguousarray(inputs[k], dtype=np.float32)
              for k in ("Wq", "Wk", "Wv", "Wo", "bo")}
    in_maps = [{"x": x[i], **shared} for i in range(b)]
    res = run_bass_kernel_spmd(nc, in_maps, list(range(b)), trace=trace)
    out = np.stack([res.results[i]["out"] for i in range(b)], axis=0)
    return out, res


def kernel(**inputs) -> np.ndarray:
    out, _ = run(inputs, trace=False)
    return out.astype(np.float32)


# revision 18
# speedup vs baseline: 1.0327x; 1.0327x over previous
"""CrissCrossAttention Trainium2 kernel (v2: all-bf16, pipelined phases).

Math notes (verified in float64): the reference's column-attention einsum
('bnjid,bnkid->bnjik' applied to grid-swapped q/k/v) is an alpha-renaming
that exactly undoes the swap, so reference == 2 * row_attention:
    out = (2 * row_attn(x)) @ Wo + bo
Row attention per (batch, head, grid-row i):
    S = Q_i K_i^T * d^-0.5 ; P = softmax_k(S) ; O_i = P V_i
with grid 64x64 (n = 4096 = i*64 + j), heads=8, d=64.

Distribution: data-parallel over batch; core b handles x[b].

v2 changes vs v1 (279us -> target ~190us):
  - whole PE datapath in bf16 (x cast once; transposes 1cyc/row; all
    LDWEIGHTS FWL-eligible 128-col or cheap 64-col)
  - pool scoping so phase-B SBUF never reuses phase-A-hot space: no
    cross-phase barrier, PE stream stays dense, HAM stays warm
  - softmax chain spread over ACT/DVE/GpSimd (P-mul on GpSimd)
  - deeper software skew (3) between S-front and O-tail
  - ot in bf16 (was f32r): final GEMM LDW 53ns instead of ~200ns
"""

import sys

if "/opt/trn_rl_repo" not in sys.path:
    sys.path.insert(0, "/opt/trn_rl_repo")

import numpy as np

import concourse.bass as bass
import concourse.mybir as mybir
import concourse.tile as tile
from concourse import bacc
from concourse.masks import make_identity

F32 = mybir.dt.float32
BF16 = mybir.dt.bfloat16

N = 4096
D = 512
G = 64          # grid side
NC = 4          # dim chunks of 128 (2 heads each)
NT = 32         # n tiles of 128
NB = 8          # n banks of 512
SCALE = G ** -0.5


def build_kernel(n_cores: int = 8):
    nc = bacc.Bacc("TRN2", target_bir_lowering=False, debug=False,
                   num_devices=n_cores)

    x_d = nc.dram_tensor("x", [N, D], F32, kind="ExternalInput").ap()
    wq_d = nc.dram_tensor("Wq", [D, D], F32, kind="ExternalInput").ap()
    wk_d = nc.dram_tensor("Wk", [D, D], F32, kind="ExternalInput").ap()
    wv_d = nc.dram_tensor("Wv", [D, D], F32, kind="ExternalInput").ap()
    wo_d = nc.dram_tensor("Wo", [D, D], F32, kind="ExternalInput").ap()
    bo_d = nc.dram_tensor("bo", [D], F32, kind="ExternalInput").ap()
    out_d = nc.dram_tensor("out", [N, D], F32, kind="ExternalOutput").ap()

    with tile.TileContext(nc) as tc:
        with (
            tc.tile_pool(name="consts", bufs=1) as consts,
            tc.tile_pool(name="qk", bufs=1) as qkpool,
            tc.tile_pool(name="vpool", bufs=1) as vpool,
            tc.tile_pool(name="wbf", bufs=1) as wbf,
        ):
            ident_bf = consts.tile([128, 128], BF16, tag="idb")
            make_identity(nc, ident_bf)

            qT = [qkpool.tile([128, N], BF16, tag=f"qT{c}", name=f"qT{c}")
                  for c in range(NC)]
            kT = [qkpool.tile([128, N], BF16, tag=f"kT{c}", name=f"kT{c}")
                  for c in range(NC)]
            v_sb = vpool.tile([128, NT, D], BF16, tag="v")

            wq_bf = wbf.tile([128, NC, D], BF16, tag="wq")
            wk_bf = wbf.tile([128, NC, D], BF16, tag="wk")
            wv_bf = wbf.tile([128, NC, D], BF16, tag="wv")
            wo_bf = wbf.tile([128, NC, D], BF16, tag="wo")
            bo128 = wbf.tile([128, D], F32, tag="bo")

            # ---- phase A: x load/cast/transpose + projections ----------
            with (
                tc.tile_pool(name="xt", bufs=1) as xtpool,
            ):
                xT = xtpool.tile([128, NC, N], BF16, tag="xT")

                with (
                    tc.tile_pool(name="wraw", bufs=1) as wraw,
                    tc.tile_pool(name="xin", bufs=6) as xin,
                    tc.tile_pool(name="xbf", bufs=6) as xbfp,
                    tc.tile_pool(name="psum0", bufs=1, space="PSUM") as psum,
                ):
                    # weights: fp32 staging on the fast sync HWDGE queue,
                    # issued before the x tiles; bf16 casts are emitted
                    # mid-queue (not head-of-line) just before first use.
                    # W staging rides the SAME sync queue as the x tiles,
                    # interleaved between banks: the DMA engines serve
                    # issue-order, so arrival order matches consumption
                    # order (tiles b0, wq, tiles b1, wk, ...). A separate
                    # queue round-robins 1MB W chunks between x tiles and
                    # starves the transpose pipeline for ~20us.
                    wstages = [wraw.tile([128, NC, D], F32, tag=f"wstage{i}",
                                         name=f"wstage{i}")
                               for i in range(4)]
                    wds = (wq_d, wk_d, wv_d, wo_d)

                    def w_load(i):
                        nc.sync.dma_start(
                            out=wstages[i],
                            in_=wds[i].rearrange("(kc p) e -> p kc e", p=128))

                    # x: load fp32 -> cast bf16 -> PE transpose -> xT
                    def load_bank(nb):
                        for nt in range(nb * 4, nb * 4 + 4):
                            xtile = xin.tile([128, D], F32, tag="xtile",
                                             name=f"xtile{nt}")
                            nc.sync.dma_start(
                                out=xtile,
                                in_=x_d[nt * 128:(nt + 1) * 128, :])
                            xb = xbfp.tile([128, D], BF16, tag="xbf",
                                           name=f"xbf{nt}")
                            if nt % 2 == 0:
                                nc.vector.tensor_copy(out=xb, in_=xtile)
                            else:
                                nc.scalar.copy(out=xb, in_=xtile)
                            tp = psum.tile([128, NC, 128], BF16, tag="xtp",
                                           bufs=3, name=f"xtp{nt}")
                            for kc in range(NC):
                                nc.tensor.transpose(
                                    tp[:, kc, :],
                                    xb[:, kc * 128:(kc + 1) * 128],
                                    ident_bf)
                            for kc in range(NC):
                                if kc % 2 == 0:
                                    nc.vector.tensor_copy(
                                        out=xT[:, kc,
                                               nt * 128:(nt + 1) * 128],
                                        in_=tp[:, kc, :])
                                else:
                                    nc.scalar.copy(
                                        out=xT[:, kc,
                                               nt * 128:(nt + 1) * 128],
                                        in_=tp[:, kc, :])

                    def qk_bank(nb):
                        for wsb, dst in ((wq_bf, qT), (wk_bf, kT)):
                            for c in range(NC):
                                pj = psum.tile([128, D], F32, tag="pj",
                                               bufs=4, name=f"pj{nb}_{c}")
                                for kc in range(NC):
                                    nc.tensor.matmul(
                                        pj,
                                        wsb[:, kc, c * 128:(c + 1) * 128],
                                        xT[:, kc, nb * D:(nb + 1) * D],
                                        start=(kc == 0), stop=(kc == NC - 1))
                                if c % 2 == 0:
                                    nc.vector.tensor_copy(
                                        out=dst[c][:, nb * D:(nb + 1) * D],
                                        in_=pj)
                                else:
                                    nc.scalar.copy(
                                        out=dst[c][:, nb * D:(nb + 1) * D],
                                        in_=pj)

                    # 1-bank software skew: bank nb+1's transposes land
                    # while the PE projects bank nb; v projections at the
                    # end (their consumers run last in phase B). W loads
                    # and casts drop into the queues right when needed.
                    load_bank(0)
                    w_load(0)
                    for nb in range(NB):
                        if nb + 1 < NB:
                            load_bank(nb + 1)
                        if nb == 0:
                            w_load(1)
                            nc.vector.tensor_copy(out=wq_bf, in_=wstages[0])
                            nc.scalar.copy(out=wk_bf, in_=wstages[1])
                        if nb == 1:
                            w_load(2)
                            nc.sync.dma_start(
                                out=bo128,
                                in_=bass.AP(tensor=bo_d.tensor,
                                            offset=bo_d.offset,
                                            ap=[[0, 128], [1, D]]))
                        if nb == 2:
                            w_load(3)
                            nc.vector.tensor_copy(out=wv_bf, in_=wstages[2])
                        if nb == 4:
                            # fold the criss-cross 2x into Wo
                            nc.scalar.mul(out=wo_bf, in_=wstages[3], mul=2.0)
                        qk_bank(nb)
                    for nt in range(NT):
                        pv = psum.tile([128, D], F32, tag="pj", bufs=4,
                                       name=f"pv{nt}")
                        for kc in range(NC):
                            nc.tensor.matmul(
                                pv,
                                xT[:, kc, nt * 128:(nt + 1) * 128],
                                wv_bf[:, kc, :],
                                start=(kc == 0), stop=(kc == NC - 1))
                        if nt % 2 == 0:
                            nc.vector.tensor_copy(out=v_sb[:, nt, :],
                                                  in_=pv)
                        else:
                            nc.scalar.copy(out=v_sb[:, nt, :], in_=pv)

                # ---- phase B: attention + output projection ------------
                with (
                    tc.tile_pool(name="otpool", bufs=1) as otpool,
                    tc.tile_pool(name="attn", bufs=1) as attn,
                    tc.tile_pool(name="outsb", bufs=2) as outp,
                    tc.tile_pool(name="psumA", bufs=1, space="PSUM") as psumb,
                ):
                    ot = [otpool.tile([128, N], BF16, tag=f"ot{c}",
                                      name=f"ot{c}")
                          for c in range(NC)]

                    pbs = {}

                    def front(bk, c):
                        """S matmuls + softmax chain; result in pbs."""
                        sbt = psumb.tile([128, 8, G], F32, tag="sb", bufs=3,
                                         name=f"sb{bk}_{c}")
                        for s in range(8):
                            i = bk * 8 + s
                            for h in range(2):
                                p0 = h * 64
                                nc.tensor.matmul(
                                    sbt[p0:p0 + 64, s, :],
                                    qT[c][p0:p0 + 64, i * G:(i + 1) * G],
                                    kT[c][p0:p0 + 64, i * G:(i + 1) * G],
                                    start=True, stop=True,
                                    tile_position=(p0, p0))
                        eb = attn.tile([128, 8, G], BF16, tag="eb", bufs=4,
                                       name=f"eb{bk}_{c}")
                        nc.scalar.activation(
                            out=eb, in_=sbt,
                            func=mybir.ActivationFunctionType.Exp,
                            scale=SCALE)
                        sums = attn.tile([128, 8], F32, tag="sums", bufs=4,
                                         name=f"sums{bk}_{c}")
                        nc.vector.reduce_sum(out=sums, in_=eb,
                                             axis=mybir.AxisListType.X)
                        rec = attn.tile([128, 8], F32, tag="rec", bufs=4,
                                        name=f"rec{bk}_{c}")
                        nc.vector.reciprocal(out=rec, in_=sums)
                        rec_b = bass.AP(tensor=rec.tensor, offset=rec.offset,
                                        ap=[rec.ap[0], rec.ap[1], [0, G]])
                        pb = attn.tile([128, 8, G], BF16, tag="pb", bufs=4,
                                       name=f"pb{bk}_{c}")
                        # last units: DVE mul (faster) shortens the drain
                        # chain at the end of the kernel when nothing is
                        # left to hide it behind
                        if bk * NC + c >= NB * NC - 4:
                            nc.vector.tensor_mul(pb, eb, rec_b)
                        else:
                            nc.gpsimd.tensor_mul(pb, eb, rec_b)
                        pbs[(bk, c)] = pb

                    def tail(bk, c):
                        """P transpose, O matmuls, extract into OT."""
                        pb = pbs.pop((bk, c))
                        ptp = psumb.tile([128, 4, 128], BF16, tag="ptp",
                                         bufs=2, name=f"ptp{bk}_{c}")
                        for p2 in range(4):
                            nc.tensor.transpose(
                                ptp[:, p2, :], pb[:, 2 * p2:2 * p2 + 2, :],
                                ident_bf)
                        pts = attn.tile([128, 4, 128], BF16, tag="pts",
                                        bufs=4, name=f"pts{bk}_{c}")
                        if c % 2 == 0:
                            nc.vector.tensor_copy(out=pts, in_=ptp)
                        else:
                            nc.scalar.copy(out=pts, in_=ptp)

                        ob = psumb.tile([128, 2, 4, 128], F32, tag="ob",
                                        name=f"ob{bk}_{c}", bufs=1)
                        for p2 in range(4):
                            for e in range(2):
                                i = bk * 8 + 2 * p2 + e
                                nc.tensor.matmul(
                                    ob[:, e, p2, :],
                                    v_sb[e * 64:e * 64 + 64, i // 2,
                                         c * 128:(c + 1) * 128],
                                    pts[e * 64:e * 64 + 64, p2, :],
                                    start=True, stop=True,
                                    tile_position=(e * 64, 0))
                        # extract diagonal (per-head) blocks into OT
                        pstride = ot[c].ap[0][0]
                        for h in range(2):
                            src_ap = ob[h * 64:h * 64 + 64, :, :,
                                        h * 64:h * 64 + 64]
                            dst = bass.AP(
                                tensor=ot[c].tensor,
                                offset=(ot[c].offset + h * 64 * pstride
                                        + bk * 8 * G),
                                ap=[[pstride, 64], [G, 2], [2 * G, 4],
                                    [1, G]])
                            if h == 1:
                                nc.scalar.copy(out=dst, in_=src_ap)
                            else:
                                nc.vector.tensor_copy(out=dst, in_=src_ap)

                    def final_bank(bk):
                        """Output projection for this bank's 4 n-tiles."""
                        for nt in range(bk * 4, bk * 4 + 4):
                            fp = psumb.tile([128, D], F32, tag="fp", bufs=1,
                                            name=f"fp{nt}")
                            for c in range(NC):
                                nc.tensor.matmul(
                                    fp, ot[c][:, nt * 128:(nt + 1) * 128],
                                    wo_bf[:, c, :],
                                    start=(c == 0), stop=(c == NC - 1))
                            osb = outp.tile([128, D], F32, tag="osb",
                                            name=f"osb{nt}")
                            nc.vector.tensor_add(osb, fp, bo128)
                            nc.sync.dma_start(
                                out=out_d[nt * 128:(nt + 1) * 128, :],
                                in_=osb)

                    # software skew: S-matmuls run SKEW units ahead of the
                    # P-transpose/O-matmul tail so the PE never stalls on
                    # the ACT/DVE/GpSimd softmax chain
                    SKEW = 3
                    units = [(bk, c) for bk in range(NB) for c in range(NC)]
                    for idx in range(len(units) + SKEW):
                        if idx < len(units):
                            front(*units[idx])
                        if idx >= SKEW:
                            bk, c = units[idx - SKEW]
                            tail(bk, c)
                            if c == NC - 1:
                                final_bank(bk)

    nc.compile()
    return nc


_CACHED = None


def _get_nc():
    global _CACHED
    if _CACHED is None:
        _CACHED = build_kernel()
    return _CACHED


def run(inputs: dict, trace: bool = False):
    from concourse.bass_utils import run_bass_kernel_spmd
    nc = _get_nc()
    x = np.ascontiguousarray(inputs["x"], dtype=np.float32)
    b = x.shape[0]
    shared = {k: np.ascontiguousarray(inputs[k], dtype=np.float32)
              for k in ("Wq", "Wk", "Wv", "Wo", "bo")}
    in_maps = [{"x": x[i], **shared} for i in range(b)]
    res = run_bass_kernel_spmd(nc, in_maps, list(range(b)), trace=trace)
    out = np.stack([res.results[i]["out"] for i in range(b)], axis=0)
    return out, res


def kernel(**inputs) -> np.ndarray:
    out, _ = run(inputs, trace=False)
    return out.astype(np.float32)
